# revision 1
# baseline (speedup 1.0000x reference)
"""Multi-head attention Trainium2 kernel, 8-core SPMD.

Sharding: 16 (batch, head) pairs over 8 cores -> each core computes 2 heads
of one batch and returns a partial [N, D] output; host sums 4 partials per
batch.

Per-core dataflow (all layouts transposed, q/m on free dims so softmax'
normalization can be deferred):
  XT = dma-transpose(x)                [D, N]  bf16 (xbar transpose needs 2B)
  QT/KT/VT = W.T @ XT                  [2*HS, N] per head pair (scale folded
                                       into Wq on host)
  S^T[m,q] = KT_h.T @ QT_h             PSUM fp32, per m-chunk of 128
  P^T = exp(S^T)                       ACT, -> SBUF bf16 (no max subtraction:
                                       logits are O(6) by construction)
  O^T[65,q] = [V_h | 1].T @ P^T        PSUM accumulate over m; row 64 = row
                                       sums r[q] (ones-column trick)
  U = O^T -> SBUF; Un = U[0:64] / r    (recip + partition broadcast)
  out[q,:] += Un_h.T @ Wp_h            accumulated over both heads in PSUM
"""

import os
import sys

import numpy as np

sys.path.insert(0, "/opt/trn_rl_repo")

import ml_dtypes
from contextlib import ExitStack

import concourse.bass as bass
import concourse.mybir as mybir
import concourse.tile as tile
from concourse import bacc
from concourse.bass_utils import run_bass_kernel_spmd
from concourse.masks import make_identity

B, N, D, H, HS = 2, 2048, 512, 8, 64
NCORES = 8
BF16 = mybir.dt.bfloat16
FP32 = mybir.dt.float32
nbf16 = ml_dtypes.bfloat16

DC = D // 128  # 4 d-chunks
MC = N // 128  # 16 m-chunks
QH = 2  # q halves
QW = N // QH  # 1024 q per chunk


def build_nc(finalize=True, repeat=1):
    nc = bacc.Bacc()
    xq = nc.dram_tensor("xq", [N, D], BF16, kind="ExternalInput")
    xk = nc.dram_tensor("xk", [N, D], BF16, kind="ExternalInput")
    xv = nc.dram_tensor("xv", [N, D], BF16, kind="ExternalInput")
    wq = nc.dram_tensor("wq", [D, 128], BF16, kind="ExternalInput")
    wk = nc.dram_tensor("wk", [D, 128], BF16, kind="ExternalInput")
    wv = nc.dram_tensor("wv", [D, 128], BF16, kind="ExternalInput")
    wp = nc.dram_tensor("wp", [2 * HS, D], BF16, kind="ExternalInput")
    out = nc.dram_tensor("out", [N, D], FP32, kind="ExternalOutput")

    with tile.TileContext(nc) as tc, ExitStack() as ctx:
        consts = ctx.enter_context(tc.tile_pool(name="consts", bufs=1))
        xt_pool = ctx.enter_context(tc.tile_pool(name="xt", bufs=1))
        proj_pool = ctx.enter_context(tc.tile_pool(name="proj", bufs=1))
        pt_pool = ctx.enter_context(tc.tile_pool(name="pt", bufs=6))
        u_pool = ctx.enter_context(tc.tile_pool(name="u", bufs=4))
        un_pool = ctx.enter_context(tc.tile_pool(name="un", bufs=4))
        rb_pool = ctx.enter_context(tc.tile_pool(name="rb", bufs=2))
        ob_pool = ctx.enter_context(tc.tile_pool(name="ob", bufs=3))
        psA = ctx.enter_context(tc.tile_pool(name="psA", bufs=2, space="PSUM"))
        psO = ctx.enter_context(tc.tile_pool(name="psO", bufs=2, space="PSUM"))

        for _rep in range(repeat):
            ident = consts.tile([128, 128], BF16)
            make_identity(nc, ident[:])

            # weights
            wq_s = consts.tile([128, DC, 128], BF16, tag="wq_s")
            wk_s = consts.tile([128, DC, 128], BF16, tag="wk_s")
            wv_s = consts.tile([128, DC, 128], BF16, tag="wv_s")
            for w_s, w_d in ((wq_s, wq), (wk_s, wk), (wv_s, wv)):
                nc.sync.dma_start(
                    out=w_s[:], in_=w_d.rearrange("(c p) h -> p c h", p=128)
                )
            wp_s = consts.tile([2 * HS, D], BF16, tag="wp_s")
            nc.sync.dma_start(out=wp_s[:], in_=wp[:])

            # Vn: [128, head, mc, 65]; col 64 = ones (rowsum trick)
            vn = consts.tile([128, 2, MC, HS + 1], BF16, tag="vn")
            nc.gpsimd.memset(vn[:, :, :, HS : HS + 1], 1.0)
            # lhsT/rhs must share a base partition; the rowsum row lives at
            # partition HS, so put the ones row there too
            ones_row = consts.tile([HS + 1, HS], BF16, tag="ones_row")
            nc.gpsimd.memset(ones_row[HS : HS + 1, :], 1.0)

            # X transposed: [128, dc, N] per tensor via ONE xbar dma transpose
            # (3D out AP: transposed row d lands at partition d%128, chunk
            # d//128 — same layout as per-chunk transposes, one DMA wait).
            xts = {}
            for name, dram in (("q", xq), ("k", xk), ("v", xv)):
                xts[name] = xt_pool.tile(
                    [128, DC, N], BF16, tag=f"xt_{name}", name=f"xt_{name}"
                )
            # halves DMA'd separately, ordered so the first attention chunk
            # (q half 0 + k half 0) is gated by as little DMA as possible
            for name, half in (
                ("k", 0),
                ("q", 0),
                ("k", 1),
                ("v", 0),
                ("v", 1),
                ("q", 1),
            ):
                dram = {"q": xq, "k": xk, "v": xv}[name]
                nc.sync.dma_start(
                    out=xts[name][:, :, half * QW : (half + 1) * QW],
                    in_=dram[half * QW : (half + 1) * QW, :],
                    transpose=True,
                )

            # projections: [2*HS, N] = sum_dc W[dc].T @ XT[dc]
            wmap = {"q": wq_s, "k": wk_s, "v": wv_s}
            projT = {}
            for name in ("q", "k", "v"):
                projT[name] = proj_pool.tile(
                    [128, N], BF16, tag=f"projT_{name}", name=f"projT_{name}"
                )

            def emit_proj_half(name, half):
                ps = psA.tile([128, QW], FP32, tag="ps", name="ps")
                for sl in range(QW // 512):
                    for dc in range(DC):
                        nc.tensor.matmul(
                            ps[:, sl * 512 : (sl + 1) * 512],
                            wmap[name][:, dc, :],
                            xts[name][
                                :,
                                dc,
                                half * QW + sl * 512 : half * QW + (sl + 1) * 512,
                            ],
                            start=(dc == 0),
                            stop=(dc == DC - 1),
                        )
                nc.vector.tensor_copy(
                    projT[name][:, half * QW : (half + 1) * QW], ps[:]
                )

            def emit_vn_block():
                # V natural: transpose VT2 per m-chunk -> [m, V_h0 | V_h1]
                for mc in range(MC):
                    pst = psA.tile([128, 128], BF16, tag="ps", name="pst")
                    nc.tensor.transpose(
                        pst[:], projT["v"][:, mc * 128 : (mc + 1) * 128], ident[:]
                    )
                    nc.vector.tensor_copy(
                        vn[:, :, mc, 0:HS],
                        pst[:].rearrange("p (b c) -> p b c", b=2),
                    )

            # only q-half-0 and k gate the first attention chunk; the rest
            # is emitted inside the first m-loop to overlap with exp waits
            emit_proj_half("k", 0)
            emit_proj_half("q", 0)
            deferred = [
                lambda: emit_proj_half("k", 1),
                lambda: emit_proj_half("v", 0),
                lambda: emit_proj_half("v", 1),
                emit_vn_block,
                lambda: emit_proj_half("q", 1),
            ]

            # attention + output projection — both heads' m-loops
            # interleaved so the PE stream stays dense (holds HAM warm)
            qt2, kt2 = projT["q"], projT["k"]
            for qh in range(QH):
                un2 = un_pool.tile([128, QW], BF16, tag="un")
                o_ps = {}
                for hh in range(2):
                    o_ps[hh] = psO.tile(
                        [HS + 1, QW], FP32, tag="o", name=f"o_ps{hh}"
                    )

                def pv(hh, j, p_sb):
                    for sl in range(QW // 512):
                        nc.tensor.matmul(
                            o_ps[hh][:, sl * 512 : (sl + 1) * 512],
                            vn[:, hh, j, :],
                            p_sb[:, sl * 512 : (sl + 1) * 512],
                            start=(j == 0),
                            stop=(j == MC - 1),
                        )

                pend = []
                for mc in range(MC):
                    for hh in range(2):
                        hs0 = HS * hh
                        s_ps = psA.tile([128, QW], FP32, tag="ps", name="s_ps")
                        for sl in range(QW // 512):
                            nc.tensor.matmul(
                                s_ps[:, sl * 512 : (sl + 1) * 512],
                                kt2[hs0 : hs0 + HS, mc * 128 : (mc + 1) * 128],
                                qt2[
                                    hs0 : hs0 + HS,
                                    qh * QW + sl * 512 : qh * QW + (sl + 1) * 512,
                                ],
                                start=True,
                                stop=True,
                            )
                        p_sb = pt_pool.tile([128, QW], BF16, tag="p", name="p_sb")
                        nc.scalar.activation(
                            p_sb[:], s_ps[:], mybir.ActivationFunctionType.Exp
                        )
                        if deferred:
                            deferred.pop(0)()
                        pend.append((hh, mc, p_sb))
                        lag = 5 if mc < MC - 1 else 1
                        while len(pend) > lag:
                            pv(*pend.pop(0))
                for e in pend:
                    pv(*e)
                def emit_uchain(hh, o_ps_=None, un2_=None):
                    o_ps_ = o_ps_ if o_ps_ is not None else o_ps
                    un2_ = un2_ if un2_ is not None else un2
                    u = u_pool.tile([HS + 1, QW], BF16, tag="u", name="u")
                    nc.vector.tensor_copy(u[:], o_ps_[hh][:])
                    # broadcast row sums r to 64 partitions via ones.T @ r
                    # (rb reuses the freed o_ps slot)
                    rb_ps = psO.tile([HS, QW], FP32, tag="o", name="rb_ps")
                    for sl in range(QW // 512):
                        nc.tensor.matmul(
                            rb_ps[:, sl * 512 : (sl + 1) * 512],
                            ones_row[HS : HS + 1, :],
                            u[HS : HS + 1, sl * 512 : (sl + 1) * 512],
                            start=True,
                            stop=True,
                        )
                    rb = rb_pool.tile([HS, QW], FP32, tag="rb", name="rb")
                    nc.vector.reciprocal_approx_fast(rb[:], rb_ps[:])
                    nc.vector.tensor_mul(
                        un2_[HS * hh : HS * hh + HS, :], u[0:HS, :], rb[:]
                    )

                if qh < QH - 1:
                    # trickle the normalization chains through the next
                    # m-loop so its S^T stream isn't queued behind them
                    for hh in range(2):
                        deferred.append(
                            lambda hh_=hh, o_=o_ps, u_=un2: emit_uchain(
                                hh_, o_, u_
                            )
                        )
                else:
                    for hh in range(2):
                        emit_uchain(hh)
                # output projection: both heads stacked on 128 partitions —
                # the contraction itself performs the head sum

                def emit_final(qh_, un2_, c):
                    f_ps = psA.tile([128, D], FP32, tag="ps", name="f_ps")
                    nc.tensor.matmul(
                        f_ps[:],
                        un2_[:, c * 128 : (c + 1) * 128],
                        wp_s[:],
                        start=True,
                        stop=True,
                    )
                    ob = ob_pool.tile([128, D], FP32, tag="ob", name="ob")
                    nc.vector.tensor_copy(ob[:], f_ps[:])
                    nc.sync.dma_start(
                        out=out[
                            qh_ * QW + c * 128 : qh_ * QW + (c + 1) * 128, :
                        ],
                        in_=ob[:],
                    )

                if qh < QH - 1:
                    # trickle these through the next m-loop's spare slots
                    for c in range(QW // 128):
                        deferred.append(
                            lambda qh_=qh, un2_=un2, c_=c: emit_final(qh_, un2_, c_)
                        )
                else:
                    for c in range(QW // 128):
                        emit_final(qh, un2, c)
    if finalize:
        nc.finalize()
    return nc


_NC_CACHE = None


def _get_nc():
    global _NC_CACHE
    if _NC_CACHE is None:
        _NC_CACHE = build_nc()
    return _NC_CACHE


def make_in_maps(inputs):
    query = np.asarray(inputs["query"], np.float32)
    key = np.asarray(inputs["key"], np.float32)
    value = np.asarray(inputs["value"], np.float32)
    Wq = np.asarray(inputs["Wq"], np.float32) / np.sqrt(np.float32(HS))
    Wk = np.asarray(inputs["Wk"], np.float32)
    Wv = np.asarray(inputs["Wv"], np.float32)
    Wp = np.asarray(inputs["Wp"], np.float32)

    in_maps = []
    for c in range(NCORES):
        b = c // 4
        h0 = 2 * (c % 4)
        in_maps.append(
            {
                "xq": query[b].astype(nbf16),
                "xk": key[b].astype(nbf16),
                "xv": value[b].astype(nbf16),
                "wq": np.concatenate([Wq[h0], Wq[h0 + 1]], axis=1).astype(nbf16),
                "wk": np.concatenate([Wk[h0], Wk[h0 + 1]], axis=1).astype(nbf16),
                "wv": np.concatenate([Wv[h0], Wv[h0 + 1]], axis=1).astype(nbf16),
                "wp": np.concatenate([Wp[h0], Wp[h0 + 1]], axis=0).astype(nbf16),
            }
        )
    return in_maps


def kernel(query, key, value, Wq, Wk, Wv, Wp):
    in_maps = make_in_maps(
        dict(query=query, key=key, value=value, Wq=Wq, Wk=Wk, Wv=Wv, Wp=Wp)
    )
    nc = _get_nc()
    res = run_bass_kernel_spmd(nc, in_maps, list(range(NCORES)))
    out = np.zeros((B, N, D), np.float32)
    for c in range(NCORES):
        out[c // 4] += np.asarray(res.results[c]["out"], np.float32)
    return out


if __name__ == "__main__":
    d = np.load("/root/problem/work/ref.npz")
    got = kernel(
        d["query"], d["key"], d["value"], d["Wq"], d["Wk"], d["Wv"], d["Wp"]
    )
    exp = d["expected"]
    rel = np.linalg.norm(got - exp) / np.linalg.norm(exp)
    print("Relative error:", rel)



# revision 13
# speedup vs baseline: 1.0794x; 1.0794x over previous
"""Multi-head attention Trainium2 kernel, 8-core SPMD.

Sharding: 16 (batch, head) pairs over 8 cores -> each core computes 2 heads
of one batch and returns a partial [N, D] output (bf16); host sums 4
partials per batch in fp32.

Per-core dataflow (all layouts transposed, q/m on free dims so softmax'
normalization can be deferred):
  XT loaded directly: q/k/v are pre-transposed on host to [D, N], so the
  SBUF [128, DC, N] layout comes from a plain strided DMA (no xbar
  transpose).  Weights stream on the scalar queue in parallel.
  QT/KT/VT = W.T @ XT                  [2*HS, N] per head pair (scale folded
                                       into Wq on host)
  S^T[m,q] = KT_h.T @ QT_h             PSUM fp32, per m-chunk of 128
  P^T = exp(S^T)                       ACT, -> SBUF bf16 (no max subtraction:
                                       logits are O(6) by construction)
  O^T[65,q] = [V_h | 1].T @ P^T        PSUM accumulate over m; row 64 = row
                                       sums r[q] (ones-column trick)
  U = O^T -> SBUF; Un = U[0:64] / r    (recip + partition broadcast)
  out[q,:] += Un_h.T @ Wp_h            accumulated over both heads in PSUM

A run of identity transposes right after setup keeps the PE busy while the
first DMAs land so the p-state ramp (0.65 -> 1.2 -> 2.4 GHz after 3us of
continuous work) completes before the projection matmuls start.
"""

import os
import sys

import numpy as np

sys.path.insert(0, "/opt/trn_rl_repo")

import ml_dtypes
from contextlib import ExitStack

import concourse.bass as bass
import concourse.mybir as mybir
import concourse.tile as tile
from concourse import bacc
from concourse.bass_utils import run_bass_kernel_spmd
from concourse.masks import make_identity

B, N, D, H, HS = 2, 2048, 512, 8, 64
NCORES = 8
BF16 = mybir.dt.bfloat16
FP32 = mybir.dt.float32
nbf16 = ml_dtypes.bfloat16

DC = D // 128  # 4 d-chunks
MC = N // 128  # 16 m-chunks
QH = 2  # q halves
QW = N // QH  # 1024 q per chunk
WARMUP = 44  # identity transposes to hold the PE p-state ramp


def build_nc(finalize=True, repeat=1):
    nc = bacc.Bacc()
    # host-pre-transposed activations: [D, N]
    xqt = nc.dram_tensor("xqt", [D, N], BF16, kind="ExternalInput")
    xkt = nc.dram_tensor("xkt", [D, N], BF16, kind="ExternalInput")
    xvt = nc.dram_tensor("xvt", [D, N], BF16, kind="ExternalInput")
    wq = nc.dram_tensor("wq", [D, 128], BF16, kind="ExternalInput")
    wk = nc.dram_tensor("wk", [D, 128], BF16, kind="ExternalInput")
    wv = nc.dram_tensor("wv", [D, 128], BF16, kind="ExternalInput")
    wp = nc.dram_tensor("wp", [2 * HS, D], BF16, kind="ExternalInput")
    out = nc.dram_tensor("out", [N, D], BF16, kind="ExternalOutput")

    with tile.TileContext(nc) as tc, ExitStack() as ctx:
        consts = ctx.enter_context(tc.tile_pool(name="consts", bufs=1))
        xt_pool = ctx.enter_context(tc.tile_pool(name="xt", bufs=1))
        proj_pool = ctx.enter_context(tc.tile_pool(name="proj", bufs=1))
        pt_pool = ctx.enter_context(tc.tile_pool(name="pt", bufs=12))
        u_pool = ctx.enter_context(tc.tile_pool(name="u", bufs=4))
        un_pool = ctx.enter_context(tc.tile_pool(name="un", bufs=4))
        rb_pool = ctx.enter_context(tc.tile_pool(name="rb", bufs=2))
        ob_pool = ctx.enter_context(tc.tile_pool(name="ob", bufs=2))
        psA = ctx.enter_context(tc.tile_pool(name="psA", bufs=2, space="PSUM"))
        psO = ctx.enter_context(tc.tile_pool(name="psO", bufs=2, space="PSUM"))

        for _rep in range(repeat):
            ident = consts.tile([128, 128], BF16)
            make_identity(nc, ident[:])

            # PE p-state warm-up: identity transposes with no DMA deps
            for _w in range(WARMUP):
                warm = psA.tile([128, 128], BF16, tag="ps", name="warm")
                nc.tensor.transpose(warm[:], ident[:], ident[:])

            # weights: wk on the sync queue ahead of k0; the rest on the
            # scalar queue (ACT is idle until the first exp)
            wq_s = consts.tile([128, DC, 128], BF16, tag="wq_s")
            wk_s = consts.tile([128, DC, 128], BF16, tag="wk_s")
            wv_s = consts.tile([128, DC, 128], BF16, tag="wv_s")
            for eng, w_s, w_d in (
                (nc.sync, wk_s, wk),
                (nc.scalar, wq_s, wq),
                (nc.scalar, wv_s, wv),
            ):
                eng.dma_start(
                    out=w_s[:], in_=w_d.rearrange("(c p) h -> p c h", p=128)
                )
            wp_s = consts.tile([2 * HS, D], BF16, tag="wp_s")
            nc.scalar.dma_start(out=wp_s[:], in_=wp[:])

            # Vn: [128, head, mc, 65]; col 64 = ones (rowsum trick)
            vn = consts.tile([128, 2, MC, HS + 1], BF16, tag="vn")
            nc.gpsimd.memset(vn[:, :, :, HS : HS + 1], 1.0)
            # lhsT/rhs must share a base partition; the rowsum row lives at
            # partition HS, so put the ones row there too
            ones_row = consts.tile([HS + 1, HS], BF16, tag="ones_row")
            nc.gpsimd.memset(ones_row[HS : HS + 1, :], 1.0)

            # X loads: plain strided DMA from the pre-transposed [D, N]
            # tensors into [128, dc, N]; row d -> partition d%128, chunk
            # d//128.  Halves ordered so the first attention chunk is gated
            # by as little DMA as possible; q halves ride the scalar queue.
            xts = {}
            for name in ("q", "k", "v"):
                xts[name] = xt_pool.tile(
                    [128, DC, N], BF16, tag=f"xt_{name}", name=f"xt_{name}"
                )

            def xdma(eng, name, half):
                dram = {"q": xqt, "k": xkt, "v": xvt}[name]
                eng.dma_start(
                    out=xts[name][:, :, half * QW : (half + 1) * QW],
                    in_=dram[:, half * QW : (half + 1) * QW].rearrange(
                        "(c p) n -> p c n", p=128
                    ),
                )

            xdma(nc.sync, "k", 0)
            xdma(nc.sync, "q", 0)
            xdma(nc.scalar, "v", 0)
            xdma(nc.scalar, "k", 1)
            xdma(nc.scalar, "v", 1)
            xdma(nc.scalar, "q", 1)

            # projections: [2*HS, N] = sum_dc W[dc].T @ XT[dc], emitted per
            # 512-col slice so deferred units stay under ~1us of PE time
            wmap = {"q": wq_s, "k": wk_s, "v": wv_s}
            projT = {}
            for name in ("q", "k", "v"):
                projT[name] = proj_pool.tile(
                    [128, N], BF16, tag=f"projT_{name}", name=f"projT_{name}"
                )

            def emit_proj_sl(name, half, sl):
                ps = psA.tile([128, 512], FP32, tag="ps", name="ps")
                c0 = half * QW + sl * 512
                for dc in range(DC):
                    nc.tensor.matmul(
                        ps[:],
                        wmap[name][:, dc, :],
                        xts[name][:, dc, c0 : c0 + 512],
                        start=(dc == 0),
                        stop=(dc == DC - 1),
                    )
                nc.vector.tensor_copy(projT[name][:, c0 : c0 + 512], ps[:])

            def emit_vn_block(mc0, mc1):
                # V natural: transpose VT2 per m-chunk -> [m, V_h0 | V_h1]
                for mc in range(mc0, mc1):
                    pst = psA.tile([128, 128], BF16, tag="ps", name="pst")
                    nc.tensor.transpose(
                        pst[:], projT["v"][:, mc * 128 : (mc + 1) * 128], ident[:]
                    )
                    nc.vector.tensor_copy(
                        vn[:, :, mc, 0:HS],
                        pst[:].rearrange("p (b c) -> p b c", b=2),
                    )

            # only k half-0 and q half-0 gate the first attention chunk; the
            # rest is trickled through the m-loop to overlap with exp waits
            emit_proj_sl("k", 0, 0)
            emit_proj_sl("k", 0, 1)
            emit_proj_sl("q", 0, 0)
            emit_proj_sl("q", 0, 1)
            deferred = [
                lambda: emit_proj_sl("v", 0, 0),
                lambda: emit_proj_sl("v", 0, 1),
                lambda: emit_vn_block(0, 4),
                lambda: emit_vn_block(4, 8),
                lambda: emit_proj_sl("k", 1, 0),
                lambda: emit_proj_sl("k", 1, 1),
                lambda: emit_proj_sl("v", 1, 0),
                lambda: emit_proj_sl("v", 1, 1),
                lambda: emit_vn_block(8, 12),
                lambda: emit_vn_block(12, 16),
                lambda: emit_proj_sl("q", 1, 0),
                lambda: emit_proj_sl("q", 1, 1),
            ]

            # attention + output projection — a single flat stream over
            # (qh, mc, hh) with the PV lag carried ACROSS the qh boundary so
            # the exp pipeline never drains mid-kernel
            qt2, kt2 = projT["q"], projT["k"]

            # normalization: u copies + rowsum broadcast + reciprocal +
            # scale, all split in 512-col pieces so the final projection
            # can start before the whole row is normalized
            def emit_ucopies(o_ps_, us):
                for hh in range(2):
                    u = u_pool.tile([HS + 1, QW], BF16, tag="u", name="u")
                    nc.vector.tensor_copy(u[:], o_ps_[hh][:])
                    us[hh] = u

            def emit_norm(us, un2_):
                rbs = {}
                for hh in range(2):
                    rb_ps = psO.tile([HS, QW], FP32, tag="o", name="rb_ps")
                    for sl in range(QW // 512):
                        nc.tensor.matmul(
                            rb_ps[:, sl * 512 : (sl + 1) * 512],
                            ones_row[HS : HS + 1, :],
                            us[hh][HS : HS + 1, sl * 512 : (sl + 1) * 512],
                            start=True,
                            stop=True,
                        )
                    rb = rb_pool.tile([HS, QW], FP32, tag="rb", name="rb")
                    rbs[hh] = (rb_ps, rb)
                for piece in range(2):
                    pc = slice(piece * 512, (piece + 1) * 512)
                    for hh in range(2):
                        rb_ps, rb = rbs[hh]
                        nc.vector.reciprocal_approx_fast(rb[:, pc], rb_ps[:, pc])
                    for hh in range(2):
                        _, rb = rbs[hh]
                        nc.vector.tensor_mul(
                            un2_[HS * hh : HS * hh + HS, pc],
                            us[hh][0:HS, pc],
                            rb[:, pc],
                        )

            # output projection: both heads stacked on 128 partitions — the
            # contraction itself performs the head sum.  Chunks are copied
            # into a batched bf16 tile; one DMA per 512 rows.
            def emit_final_group(qh_, un2_, g, last):
                ob = ob_pool.tile([128, 4, 512], BF16, tag="ob", name="ob")
                for j in range(4):
                    c = 4 * g + j
                    f_ps = psA.tile([128, D], FP32, tag="ps", name="f_ps")
                    nc.tensor.matmul(
                        f_ps[:],
                        un2_[:, c * 128 : (c + 1) * 128],
                        wp_s[:],
                        start=True,
                        stop=True,
                    )
                    if last and j % 2 == 0:
                        # ACT is idle after the last exp; share with DVE
                        nc.scalar.copy(ob[:, j, :], f_ps[:])
                    else:
                        nc.vector.tensor_copy(ob[:, j, :], f_ps[:])
                base = qh_ * QW + g * 512
                nc.sync.dma_start(
                    out=out[base : base + 512, :].rearrange(
                        "(c p) d -> p c d", p=128
                    ),
                    in_=ob[:],
                )

            o_pss = {}
            un2s = {}
            n_pv = {qh: 0 for qh in range(QH)}
            tail_hold = [0]  # >0: a qh tail is mid-flight in deferred

            def alloc_o(qh_):
                o_pss[qh_] = {
                    hh: psO.tile([HS + 1, QW], FP32, tag="o", name=f"o_ps{hh}")
                    for hh in range(2)
                }

            def hold_done():
                tail_hold[0] -= 1

            def schedule_tail(qh_):
                un2 = un_pool.tile([128, QW], BF16, tag="un", name="un")
                un2s[qh_] = un2
                us = {}
                if qh_ < QH - 1:
                    # PV pops are held until these units finish so psO slots
                    # cycle o(qh) -> rb(qh) -> o(qh+1) in program order
                    tail_hold[0] += 1
                    deferred.append(
                        lambda o_=o_pss[qh_], us_=us: emit_ucopies(o_, us_)
                    )
                    deferred.append(lambda us_=us, u_=un2: emit_norm(us_, u_))
                    deferred.append(lambda qh__=qh_: alloc_o(qh__ + 1))
                    for g in range(2):
                        deferred.append(
                            lambda qh__=qh_, un2_=un2, g_=g: emit_final_group(
                                qh__, un2_, g_, False
                            )
                        )
                    deferred.append(hold_done)
                else:
                    emit_ucopies(o_pss[qh_], us)
                    emit_norm(us, un2)
                    for g in range(2):
                        emit_final_group(qh_, un2, g, True)

            def pv(qh_, hh, j, p_sb):
                o_ = o_pss[qh_]
                for sl in range(QW // 512):
                    nc.tensor.matmul(
                        o_[hh][:, sl * 512 : (sl + 1) * 512],
                        vn[:, hh, j, :],
                        p_sb[:, sl * 512 : (sl + 1) * 512],
                        start=(j == 0),
                        stop=(j == MC - 1),
                    )
                n_pv[qh_] += 1
                if n_pv[qh_] == 2 * MC:
                    schedule_tail(qh_)

            alloc_o(0)
            pend = []
            slot = 0
            for qh in range(QH):
                for mc in range(MC):
                    for hh in range(2):
                        hs0 = HS * hh
                        s_ps = psA.tile([128, QW], FP32, tag="ps", name="s_ps")
                        for sl in range(QW // 512):
                            nc.tensor.matmul(
                                s_ps[:, sl * 512 : (sl + 1) * 512],
                                kt2[hs0 : hs0 + HS, mc * 128 : (mc + 1) * 128],
                                qt2[
                                    hs0 : hs0 + HS,
                                    qh * QW + sl * 512 : qh * QW + (sl + 1) * 512,
                                ],
                                start=True,
                                stop=True,
                            )
                        p_sb = pt_pool.tile([128, QW], BF16, tag="p", name="p_sb")
                        nc.scalar.activation(
                            p_sb[:], s_ps[:], mybir.ActivationFunctionType.Exp
                        )
                        # pop deferred work 2 slots late so a unit whose DMA
                        # hasn't landed can't head-of-line block the PE queue
                        if deferred and slot >= 2:
                            fn = deferred.pop(0)
                            if fn is not None:
                                fn()
                        slot += 1
                        pend.append((qh, hh, mc, p_sb))
                        if tail_hold[0]:
                            lag = 10**9
                        elif (qh, mc) != (QH - 1, MC - 1):
                            lag = 5
                        else:
                            lag = 1
                        while len(pend) > lag:
                            pv(*pend.pop(0))
            for e in pend:
                pv(*e)
            # drain any tail work not yet popped (e.g. the last qh's units)
            while deferred:
                fn = deferred.pop(0)
                if fn is not None:
                    fn()
    if finalize:
        nc.finalize()
    return nc


_NC_CACHE = None


def _get_nc():
    global _NC_CACHE
    if _NC_CACHE is None:
        _NC_CACHE = build_nc()
    return _NC_CACHE


def make_in_maps(inputs):
    query = np.asarray(inputs["query"], np.float32)
    key = np.asarray(inputs["key"], np.float32)
    value = np.asarray(inputs["value"], np.float32)
    Wq = np.asarray(inputs["Wq"], np.float32) / np.sqrt(np.float32(HS))
    Wk = np.asarray(inputs["Wk"], np.float32)
    Wv = np.asarray(inputs["Wv"], np.float32)
    Wp = np.asarray(inputs["Wp"], np.float32)

    # per-batch transposed bf16 activations, shared by 4 cores each
    xt = {}
    for b in range(B):
        xt[b] = {
            "xqt": np.ascontiguousarray(query[b].astype(nbf16).T),
            "xkt": np.ascontiguousarray(key[b].astype(nbf16).T),
            "xvt": np.ascontiguousarray(value[b].astype(nbf16).T),
        }

    in_maps = []
    for c in range(NCORES):
        b = c // 4
        h0 = 2 * (c % 4)
        m = dict(xt[b])
        m.update(
            {
                "wq": np.concatenate([Wq[h0], Wq[h0 + 1]], axis=1).astype(nbf16),
                "wk": np.concatenate([Wk[h0], Wk[h0 + 1]], axis=1).astype(nbf16),
                "wv": np.concatenate([Wv[h0], Wv[h0 + 1]], axis=1).astype(nbf16),
                "wp": np.concatenate([Wp[h0], Wp[h0 + 1]], axis=0).astype(nbf16),
            }
        )
        in_maps.append(m)
    return in_maps


def kernel(query, key, value, Wq, Wk, Wv, Wp):
    in_maps = make_in_maps(
        dict(query=query, key=key, value=value, Wq=Wq, Wk=Wk, Wv=Wv, Wp=Wp)
    )
    nc = _get_nc()
    res = run_bass_kernel_spmd(nc, in_maps, list(range(NCORES)))
    out = np.zeros((B, N, D), np.float32)
    for c in range(NCORES):
        out[c // 4] += np.asarray(res.results[c]["out"], np.float32)
    return out


if __name__ == "__main__":
    d = np.load("/root/problem/work/ref.npz")
    got = kernel(
        d["query"], d["key"], d["value"], d["Wq"], d["Wk"], d["Wv"], d["Wp"]
    )
    exp = d["expected"]
    rel = np.linalg.norm(got - exp) / np.linalg.norm(exp)
    print("Relative error:", rel)
